# revision 34
# baseline (speedup 1.0000x reference)
"""CrossAttention Trainium2 kernel (8-core data-parallel over batch).

Reference computation (per batch b, head h):
  rv_lat = x_lat @ W_rv_lat -> r_lat, v_lat [B,H,64,32]
  rv_pat = x_pat @ W_rv_pat -> r_pat, v_pat [B,H,4096,32]
  scores = r_lat @ r_pat^T * scale          [64, 4096]
  out_lat = rowsoftmax(scores) @ v_pat -> proj -> [B,64,192]
  out_pat = colsoftmax(scores)^T @ v_lat -> proj -> [B,4096,192]

Strategy per core (4 batches):
  - x_pat transposed to feature-major via PE transposes (bf16)
  - rv_pat^T computed feature-major on PE (bf16)
  - scores in both orientations on PE; exp fused into PSUM->SBUF
    eviction on ACT (no max subtraction: |scale*scores| < ~3)
  - colsum via ones-column in the U matmul; rowsum via ones-column in
    the out_lat matmul; normalization with per-partition reciprocals
  - out_pat projection consumes U^T (PE-transposed) as PE weights,
    bias folded in via an ones-row in the K dimension
"""

import sys

sys.path.insert(0, "/opt/trn_rl_repo")

import numpy as np

B, N_LAT, N_PAT = 32, 64, 4096
D = 192          # DIM_LAT == DIM_PAT == DIM_ATTN
H, HD = 6, 32
D2 = 2 * D       # rv width 384
SCALE = HD ** -0.5
N_CORES = 8
B_LOC = B // N_CORES  # 4 batches per core

_CACHE = {}


def _build():
    import concourse.bass as bass
    import concourse.mybir as mybir
    import concourse.tile as tile
    from concourse import bacc
    from concourse.masks import make_identity

    f32 = mybir.dt.float32
    bf16 = mybir.dt.bfloat16
    EXP = mybir.ActivationFunctionType.Exp

    nc = bacc.Bacc()

    xlat_d = nc.dram_tensor("x_latents", [B_LOC, N_LAT, D], f32, kind="ExternalInput")
    xpat_d = nc.dram_tensor("x_patches", [B_LOC, N_PAT, D], f32, kind="ExternalInput")
    wrl_d = nc.dram_tensor("W_rv_lat", [D, D2], f32, kind="ExternalInput")
    wrp_d = nc.dram_tensor("W_rv_pat", [D, D2], f32, kind="ExternalInput")
    wpl_d = nc.dram_tensor("W_proj_lat", [D, D], f32, kind="ExternalInput")
    bpl_d = nc.dram_tensor("b_proj_lat", [D], f32, kind="ExternalInput")
    wpp_d = nc.dram_tensor("W_proj_pat", [D, D], f32, kind="ExternalInput")
    bpp_d = nc.dram_tensor("b_proj_pat", [D], f32, kind="ExternalInput")
    olat_d = nc.dram_tensor("out_lat", [B_LOC, N_LAT, D], f32, kind="ExternalOutput")
    opat_d = nc.dram_tensor("out_pat", [B_LOC, N_PAT, D], f32, kind="ExternalOutput")

    with tile.TileContext(nc) as tc:
        with (
            tc.tile_pool(name="wpool", bufs=1) as wp,
            tc.tile_pool(name="big", bufs=1) as bigp,
            tc.tile_pool(name="io", bufs=2) as iop,
            tc.tile_pool(name="psA", bufs=2, space="PSUM") as psA,
            tc.tile_pool(name="psU", bufs=2, space="PSUM") as psU,
            tc.tile_pool(name="psS", bufs=2, space="PSUM") as psS,
        ):
            # ---------------- stage 0: constants ----------------
            ident_bf = wp.tile([128, 128], bf16)
            make_identity(nc, ident_bf)
            ident_f = wp.tile([128, 128], f32)
            make_identity(nc, ident_f)

            def load_cvt(dram_ap, shape, tag):
                t_f = iop.tile(shape, f32, tag="wtmp")
                nc.sync.dma_start(out=t_f[:, :], in_=dram_ap)
                t_b = wp.tile(shape, bf16, tag=tag)
                nc.vector.tensor_copy(t_b[:, :], t_f[:, :])
                return t_b

            wrp_hi = load_cvt(wrp_d[0:128, :], [128, D2], "wrp_hi")
            wrp_lo = load_cvt(wrp_d[128:D, :], [64, D2], "wrp_lo")
            wrl_hi = load_cvt(wrl_d[0:128, :], [128, D2], "wrl_hi")
            wrl_lo = load_cvt(wrl_d[128:D, :], [64, D2], "wrl_lo")
            wpl_hi = load_cvt(wpl_d[0:128, :], [128, D], "wpl_hi")
            wpl_lo = load_cvt(wpl_d[128:D, :], [64, D], "wpl_lo")
            wpp_hi = load_cvt(wpp_d[0:128, :], [128, D], "wpp_hi")
            # rows 0:64 = W_proj_pat[128:192], row 64 = b_proj_pat
            wpp_lo = wp.tile([65, D], bf16)
            tmp_w = iop.tile([65, D], f32, tag="wtmp2")
            nc.sync.dma_start(out=tmp_w[0:64, :], in_=wpp_d[128:D, :])
            nc.sync.dma_start(out=tmp_w[64:65, :], in_=bpp_d[None, :])
            nc.vector.tensor_copy(wpp_lo[:, :], tmp_w[:, :])
            # lat bias as per-partition columns
            bl_hi = wp.tile([128, 1], f32)
            nc.sync.dma_start(out=bl_hi[:, :], in_=bpl_d[0:128][:, None])
            bl_lo = wp.tile([64, 1], f32)
            nc.sync.dma_start(out=bl_lo[:, :], in_=bpl_d[128:D][:, None])

            for b in range(B_LOC):
                # ---------------- stage A: latents ----------------
                xl_f = iop.tile([64, D], f32, tag="xl_f")
                nc.sync.dma_start(out=xl_f[:, :], in_=xlat_d[b])
                xl_b = iop.tile([64, D], bf16, tag="xl_b")
                nc.vector.tensor_copy(xl_b[:, :], xl_f[:, :])

                ps_t1 = psS.tile([128, 64], bf16, tag="sm")
                nc.tensor.transpose(ps_t1[:, :], xl_b[:, 0:128], ident_bf[0:64, 0:64])
                xlT_hi = bigp.tile([128, 64], bf16, tag="xlT_hi")
                nc.vector.tensor_copy(xlT_hi[:, :], ps_t1[:, :])
                ps_t2 = psS.tile([64, 64], bf16, tag="sm")
                nc.tensor.transpose(ps_t2[:, :], xl_b[:, 128:D], ident_bf[0:64, 0:64])
                xlT_lo = bigp.tile([64, 64], bf16, tag="xlT_lo")
                nc.vector.tensor_copy(xlT_lo[:, :], ps_t2[:, :])

                # r_lat^T feature-major, two 128-col blocks of W_rv_lat
                rlT = []
                for blk in range(2):
                    ps = psU.tile([128, 64], f32, tag="u5")
                    nc.tensor.matmul(ps[:, :], wrl_hi[:, 128 * blk:128 * blk + 128],
                                     xlT_hi[:, :], start=True, stop=False)
                    nc.tensor.matmul(ps[:, :], wrl_lo[:, 128 * blk:128 * blk + 128],
                                     xlT_lo[:, :], start=False, stop=True)
                    t = bigp.tile([128, 64], bf16, tag=f"rlT{blk}")
                    nc.vector.tensor_copy(t[:, :], ps[:, :])
                    rlT.append(t)

                # rv_lat natural orientation for v_lat (+ ones col), rows
                # duplicated to partitions 64:128 for row-group packing
                ps_rvn = psU.tile([64, D2], f32, tag="u5")
                nc.tensor.matmul(ps_rvn[:, :], xlT_hi[:, :], wrl_hi[:, :],
                                 start=True, stop=False)
                nc.tensor.matmul(ps_rvn[:, :], xlT_lo[:, :], wrl_lo[:, :],
                                 start=False, stop=True)
                vl = bigp.tile([64, H, 33], bf16, tag="vl")
                nc.vector.tensor_copy(
                    vl[:, :, 0:32],
                    ps_rvn[:, D:D2].rearrange("p (h d) -> p h d", h=H))
                nc.vector.memset(vl[:, :, 32:33], 1.0)

                # block-diagonal r_lat^T tiles (2 heads each), base partition 0
                bdA = []
                for g in range(3):
                    t = bigp.tile([64, 128], bf16, tag=f"bd{g}")
                    nc.vector.memset(t[:, :], 0.0)
                    src = rlT[0] if g < 2 else rlT[1]
                    r0 = (2 * g) % 4 * 32
                    nc.sync.dma_start(out=t[0:32, 0:64],
                                      in_=src[r0:r0 + 32, :])
                    nc.sync.dma_start(out=t[32:64, 64:128],
                                      in_=src[r0 + 32:r0 + 64, :])
                    bdA.append(t)


                # ---------------- stage B: patches ----------------
                # x^T via PE transposes, bank-packed 4-wide before eviction
                xpT_hi = bigp.tile([128, N_PAT], bf16, tag="bigT1")
                xpT_lo = bigp.tile([128, N_PAT], bf16, tag="bigT2")
                for s in range(8):
                    xp_f = iop.tile([128, 4, D], f32, tag="xp_f")
                    nc.sync.dma_start(
                        out=xp_f[:, :, :],
                        in_=xpat_d[b, 512 * s:512 * (s + 1), :]
                        .rearrange("(q p) d -> p q d", p=128))
                    xp_b = iop.tile([128, 4, D], bf16, tag="xp_b")
                    nc.gpsimd.tensor_copy(xp_b[:, :, :], xp_f[:, :, :])
                    ps1 = psA.tile([128, 4, 128], bf16, tag="sc")
                    ps2 = psA.tile([64, 4, 128], bf16, tag="sc")
                    for q in range(4):
                        nc.tensor.transpose(ps1[:, q, :], xp_b[:, q, 0:128],
                                            ident_bf[:, :])
                        nc.tensor.transpose(ps2[:, q, :], xp_b[:, q, 128:D],
                                            ident_bf[:, :])
                    cs = slice(512 * s, 512 * (s + 1))
                    nc.vector.tensor_copy(xpT_hi[:, cs],
                                          ps1[:, :, :].rearrange("p a c -> p (a c)"))
                    nc.scalar.copy(xpT_lo[0:64, cs],
                                   ps2[:, :, :].rearrange("p a c -> p (a c)"))

                # rv_pat^T feature-major (3 blocks of 128 output features)
                rT_a = bigp.tile([128, N_PAT], bf16, tag="rT_a")   # r heads 0-3
                rT23 = bigp.tile([64, N_PAT], bf16, tag="rT23")    # base-0 copy of h2,h3
                rT_b = bigp.tile([64, N_PAT], bf16, tag="rT_b")    # r heads 4,5
                for c in range(8):
                    cs = slice(512 * c, 512 * (c + 1))
                    for blk in range(2):
                        ps = psU.tile([128, 512], f32, tag="u5")
                        nc.tensor.matmul(ps[:, :], wrp_hi[:, 128 * blk:128 * blk + 128],
                                         xpT_hi[:, cs], start=True, stop=False)
                        nc.tensor.matmul(ps[:, :], wrp_lo[:, 128 * blk:128 * blk + 128],
                                         xpT_lo[0:64, cs], start=False, stop=True)
                        if blk == 0:
                            nc.vector.tensor_copy(rT_a[:, cs], ps[:, :])
                            nc.sync.dma_start(out=rT23[:, cs],
                                              in_=rT_a[64:128, cs])
                        else:
                            nc.vector.tensor_copy(rT_b[:, cs], ps[0:64, :])

                # v_pat natural [pat, head*32] via a second matmul in the
                # natural orientation (xbar transposes are queue-limited)
                v_nat = bigp.tile([128, 32, H * 32], bf16, tag="v_nat")
                for cc in range(32):
                    ps = psU.tile([128, D], f32, tag="u5")
                    nc.tensor.matmul(ps[:, :],
                                     xpT_hi[:, 128 * cc:128 * cc + 128],
                                     wrp_hi[:, D:D2], start=True, stop=False)
                    nc.tensor.matmul(ps[:, :],
                                     xpT_lo[0:64, 128 * cc:128 * cc + 128],
                                     wrp_lo[:, D:D2], start=False, stop=True)
                    nc.vector.tensor_copy(v_nat[:, cc, :], ps[:, :])

                # scores-A + exp (E_A[lat(2 heads), pat] per head-group).
                # accum_out gives partial row-sums over pat for free.
                E_A = bigp.tile([128, 3, N_PAT], bf16, tag="E_A")
                rs_parts = iop.tile([128, 3, 4], f32, tag="rs_parts")
                rT2h = [rT_a, rT23, rT_b]  # 2-head slices, all base partition 0
                for g in range(3):
                    rhs_t = rT2h[g]
                    for cp in range(4):  # pairs of 512-chunks
                        ps = psA.tile([128, 1024], f32, tag="sc")
                        for k in range(2):
                            c = 2 * cp + k
                            nc.tensor.matmul(
                                ps[:, 512 * k:512 * k + 512],
                                bdA[g][:, :],
                                rhs_t[0:64, 512 * c:512 * c + 512],
                                start=True, stop=True)
                        nc.scalar.activation(
                            E_A[:, g, 1024 * cp:1024 * (cp + 1)], ps[:, :],
                            EXP, scale=SCALE,
                            accum_out=rs_parts[:, g, cp:cp + 1])
                # row-sums per (2-head-stacked lat): [128, 3]; reciprocal
                rc_rs = iop.tile([128, 3], f32, tag="rc_rs")
                rs_full = iop.tile([128, 3], f32, tag="rs_full")
                nc.vector.tensor_reduce(rs_full[:, :], rs_parts[:, :, :],
                                        axis=mybir.AxisListType.X,
                                        op=mybir.AluOpType.add)
                nc.vector.reciprocal(rc_rs[:, :], rs_full[:, :])
                rc_odd = iop.tile([64, 3], f32, tag="rc_odd")
                nc.sync.dma_start(out=rc_odd[:, :], in_=rc_rs[64:128, :])
                # base-0 copy of the odd-head (rows 64:128) halves of E_A
                E_Ao = bigp.tile([64, 3, N_PAT], bf16, tag="E_Ao")
                nc.sync.dma_start(out=E_Ao[:, :, :], in_=E_A[64:128, :, :])

                # scores-B + exp (E_B[pat, (cc, head, lat)]).
                # lhsT = 2-head feature slice of r_pat^T, rhs = the same
                # block-diagonal r_lat^T tile used for scores-A.
                E_B = bigp.tile([128, 32, H, 64], bf16, tag="E_B")
                for g in range(3):
                    lhsT_t = rT2h[g]
                    for q in range(8):  # groups of 4 128-row chunks
                        ps = psA.tile([128, 4, 2, 64], f32, tag="sc")
                        for k in range(4):
                            cc = 4 * q + k
                            nc.tensor.matmul(
                                ps[:, k, :, :],
                                lhsT_t[0:64, 128 * cc:128 * cc + 128],
                                bdA[g][:, :],
                                start=True, stop=True)
                        nc.scalar.activation(
                            E_B[:, 4 * q:4 * q + 4, 2 * g:2 * g + 2, :],
                            ps[:, :, :, :], EXP, scale=SCALE)

                # U = attn_T^T @ v_lat (unnorm, + colsum col) then normalize.
                # 8 head slots (2 junk) so the xbar transpose below can read
                # 128-wide blocks.
                U_norm = bigp.tile([128, 32, H, 32], bf16, tag="U_norm")
                for u in range(16):  # pairs of 128-row chunks
                    ps = psU.tile([128, 2, H, 33], f32, tag="u5")
                    for k in range(2):
                        cc = 2 * u + k
                        for h in range(H):
                            src = E_A if h % 2 == 0 else E_Ao
                            nc.tensor.matmul(
                                ps[:, k, h, :],
                                src[0:64, h // 2, 128 * cc:128 * cc + 128],
                                vl[:, h, :],
                                start=True, stop=True)
                    rc = iop.tile([128, 2, H], f32, tag="rc")
                    nc.vector.reciprocal(rc[:, :, :], ps[:, :, :, 32])
                    nc.vector.tensor_tensor(
                        U_norm[:, 2 * u:2 * u + 2, :, :],
                        ps[:, :, :, 0:32],
                        rc[:, :, :].to_broadcast((128, 2, H, 32)),
                        mybir.AluOpType.mult)

                # U^T feature-major via xbar; ones row (64) for bias trick.
                # UT_lo rows 65:128 receive transposed junk (never read);
                # row 64 is overwritten with ones after the transposes.
                UT_hi = bigp.tile([128, N_PAT], bf16, tag="bigT1")
                UT_lo = bigp.tile([128, N_PAT], bf16, tag="bigT2")
                for cc in range(32):
                    c0 = 128 * cc
                    ps1 = psA.tile([128, 128], bf16, tag="sc")
                    nc.tensor.transpose(ps1[:, :], U_norm[:, cc, 0:4, :],
                                        ident_bf[:, :])
                    nc.vector.tensor_copy(UT_hi[:, c0:c0 + 128], ps1[:, :])
                    ps2 = psA.tile([64, 128], bf16, tag="sc")
                    nc.tensor.transpose(ps2[:, :], U_norm[:, cc, 4:6, :],
                                        ident_bf[:, :])
                    nc.vector.tensor_copy(UT_lo[0:64, c0:c0 + 128], ps2[:, :])
                nc.vector.memset(UT_lo[64:65, :], 1.0)

                # out_pat projection (pat-major result) + store
                for pp in range(16):
                    ps = psU.tile([128, 2, D], f32, tag="u5")
                    for k in range(2):
                        cc = 2 * pp + k
                        nc.tensor.matmul(ps[:, k, :],
                                         UT_hi[:, 128 * cc:128 * cc + 128],
                                         wpp_hi[:, :], start=True, stop=False)
                        nc.tensor.matmul(ps[:, k, :],
                                         UT_lo[0:65, 128 * cc:128 * cc + 128],
                                         wpp_lo[:, :], start=False, stop=True)
                    o = iop.tile([128, 2, D], f32, tag="opat")
                    if pp % 2 == 0:
                        nc.vector.tensor_copy(o[:, :, :], ps[:, :, :])
                    else:
                        nc.scalar.copy(o[:, :, :], ps[:, :, :])
                    nc.sync.dma_start(
                        out=opat_d[b, 256 * pp:256 * (pp + 1), :]
                        .rearrange("(q p) d -> p q d", p=128),
                        in_=o[:, :, :])

                # out_lat chain
                olnT_hi = bigp.tile([128, 64], bf16, tag="olnT_hi")
                olnT_lo = bigp.tile([64, 64], bf16, tag="olnT_lo")
                for h in range(H):
                    ps_ol = psS.tile([32, 64], f32, tag="sm")
                    for cc in range(32):
                        nc.tensor.matmul(ps_ol[:, :],
                                         v_nat[:, cc, 32 * h:32 * h + 32],
                                         E_B[:, cc, h, :],
                                         start=(cc == 0), stop=(cc == 31))
                    olh = iop.tile([32, 64], f32, tag="olh")
                    nc.vector.tensor_copy(olh[:, :], ps_ol[:, :])
                    ps_olT = psS.tile([64, 32], f32, tag="sm")
                    nc.tensor.transpose(ps_olT[:, :], olh[:, :],
                                        ident_f[0:32, 0:32])
                    oln = iop.tile([64, 32], bf16, tag="oln")
                    rc_src = rc_rs if h % 2 == 0 else rc_odd
                    nc.vector.tensor_scalar_mul(
                        oln[:, :], ps_olT[:, :],
                        rc_src[0:64, h // 2:h // 2 + 1])
                    ps_ot = psS.tile([32, 64], bf16, tag="sm")
                    nc.tensor.transpose(ps_ot[:, :], oln[:, :],
                                        ident_bf[0:64, 0:64])
                    ostg = iop.tile([32, 64], bf16, tag="ostg")
                    nc.vector.tensor_copy(ostg[:, :], ps_ot[:, :])
                    if h < 4:
                        nc.sync.dma_start(out=olnT_hi[32 * h:32 * h + 32, :],
                                          in_=ostg[:, :])
                    else:
                        nc.sync.dma_start(out=olnT_lo[32 * (h - 4):32 * (h - 4) + 32, :],
                                          in_=ostg[:, :])

                # project latents: psum [192 (2 blocks), 64] feature-major
                opl = iop.tile([128, 64], f32, tag="opl")
                opl2 = iop.tile([64, 64], f32, tag="opl2")
                ps_pl1 = psS.tile([128, 64], f32, tag="sm")
                nc.tensor.matmul(ps_pl1[:, :], wpl_hi[:, 0:128], olnT_hi[:, :],
                                 start=True, stop=False)
                nc.tensor.matmul(ps_pl1[:, :], wpl_lo[:, 0:128], olnT_lo[:, :],
                                 start=False, stop=True)
                nc.vector.tensor_scalar_add(opl[:, :], ps_pl1[:, :], bl_hi[:, :])
                ps_pl2 = psS.tile([64, 64], f32, tag="sm")
                nc.tensor.matmul(ps_pl2[:, :], wpl_hi[:, 128:D], olnT_hi[:, :],
                                 start=True, stop=False)
                nc.tensor.matmul(ps_pl2[:, :], wpl_lo[:, 128:D], olnT_lo[:, :],
                                 start=False, stop=True)
                nc.vector.tensor_scalar_add(opl2[:, :], ps_pl2[:, :], bl_lo[:, :])

                ps_fl = psS.tile([64, D], f32, tag="sm")
                nc.tensor.transpose(ps_fl[:, 0:128], opl[:, 0:64],
                                    ident_f[0:128, 0:128])
                nc.tensor.transpose(ps_fl[:, 128:D], opl2[:, :],
                                    ident_f[0:64, 0:64])
                ol_out = iop.tile([64, D], f32, tag="ol_out")
                nc.vector.tensor_copy(ol_out[:, :], ps_fl[:, :])
                nc.sync.dma_start(out=olat_d[b], in_=ol_out[:, :])

    nc.compile()
    return nc


def _get_nc():
    if "nc" not in _CACHE:
        _CACHE["nc"] = _build()
    return _CACHE["nc"]


def kernel(x_latents, x_patches, W_rv_lat, W_rv_pat,
           W_proj_lat, b_proj_lat, W_proj_pat, b_proj_pat):
    from concourse.bass_utils import run_bass_kernel_spmd

    nc = _get_nc()
    x_latents = np.asarray(x_latents, dtype=np.float32)
    x_patches = np.asarray(x_patches, dtype=np.float32)
    shared = {
        "W_rv_lat": np.asarray(W_rv_lat, np.float32),
        "W_rv_pat": np.asarray(W_rv_pat, np.float32),
        "W_proj_lat": np.asarray(W_proj_lat, np.float32),
        "b_proj_lat": np.asarray(b_proj_lat, np.float32),
        "W_proj_pat": np.asarray(W_proj_pat, np.float32),
        "b_proj_pat": np.asarray(b_proj_pat, np.float32),
    }
    in_maps = []
    for c in range(N_CORES):
        sl = slice(c * B_LOC, (c + 1) * B_LOC)
        in_maps.append({"x_latents": np.ascontiguousarray(x_latents[sl]),
                        "x_patches": np.ascontiguousarray(x_patches[sl]),
                        **shared})
    res = run_bass_kernel_spmd(nc, in_maps, core_ids=list(range(N_CORES)))
    out_lat = np.concatenate([res.results[c]["out_lat"] for c in range(N_CORES)], 0)
    out_pat = np.concatenate([res.results[c]["out_pat"] for c in range(N_CORES)], 0)
    return out_lat, out_pat
